# revision 48
# baseline (speedup 1.0000x reference)
"""DeformableConv2D (DCNv2) forward on 8 Trainium2 NeuronCores.

Data-parallel over batch: one sample per core. Per core: offset conv on the
tensor engine (fp16 operands, fp32 accumulate); sampling coordinates and
bilinear weights on the vector engine; modulated bilinear sampling via SWDGE
dma_gather of 2x2-patch rows stored channel-major/corner-minor so the
bilinear-weight multiply runs in the DVE 2x fp16 mode; corner combination via
accumulating PE transposes; im2col GEMM on the tensor engine. The offset
conv / coordinate pipeline is split in halves so the gather stream starts
while the second half is still being computed.
"""
import sys
sys.path.insert(0, "/opt/trn_rl_repo")

import numpy as np
import ml_dtypes

import concourse.bass as bass
import concourse.bacc as bacc
import concourse.mybir as mybir
import concourse.tile as tile
from concourse import library_config

F32 = mybir.dt.float32
F16 = mybir.dt.float16
I16 = mybir.dt.int16
AL = mybir.AluOpType

H = W = 64
C = 128
F = 256
K = 9
PADR = 8                 # padded-coordinate margin
HP = WP = 80             # padded image
NPIX = H * W             # 4096
NBLK = 32                # pixel blocks of 128 (2 rows each)
CONVW = 66               # conv grid width (pad 1)
CONVN = 4608             # padded conv output length (9 tiles of 512)
XCLM = 67 + CONVN + 67   # xcl with shift margins
NROWS = 2 * HP * 40      # pair-table rows = 6400

DY = np.repeat(np.arange(3) - 1, 3).astype(np.float32)   # per-tap dy
DX = np.tile(np.arange(3) - 1, 3).astype(np.float32)     # per-tap dx


def bcast(ap, shape):
    return ap.to_broadcast(list(shape))


_NC = None


def build_nc():
    # 36 KB SWDGE scratch = 2304-descriptor ring: two 1152-row gathers in
    # flight, one gather per 9-slot block
    nc = bacc.Bacc("TRN2", target_bir_lowering=False,
                   dynamic_dma_scratch_size=36864)
    xcl = nc.dram_tensor("xcl", [C, XCLM], F16, kind="ExternalInput")
    pairs = nc.dram_tensor("pairs", [NROWS, 512], F16, kind="ExternalInput")
    offk = nc.dram_tensor("offk", [C, K * 27], F16, kind="ExternalInput")
    # row 0: 128 ones, then offset bias (f16) for the K=1 bias matmul
    c16r = nc.dram_tensor("c16r", [1, 160], F16, kind="ExternalInput")
    filt = nc.dram_tensor("filt", [C, K * 2 * 128], F16, kind="ExternalInput")
    eye16 = nc.dram_tensor("eye16", [128, 128], F16, kind="ExternalInput")
    # consts: Y_all [128,32], dy/dx rows [128,9] each, X_all [128,1]
    consts = nc.dram_tensor("consts", [128, 51], F32, kind="ExternalInput")
    out_d = nc.dram_tensor("out", [2, 128, NPIX], F16, kind="ExternalOutput")

    with tile.TileContext(nc) as tc:
        with (
            tc.tile_pool(name="const", bufs=1) as cpool,
            tc.tile_pool(name="bwork", bufs=1) as bpool,
            tc.tile_pool(name="conv", bufs=1) as vpool,
            tc.tile_pool(name="dram", bufs=1, space="DRAM") as dpool,
            tc.tile_pool(name="sgpool", bufs=6) as sgpool,
            tc.tile_pool(name="blkpool", bufs=4) as blkpool,
            tc.tile_pool(name="colpool", bufs=2) as colpool,
            tc.tile_pool(name="psc", bufs=2, space="PSUM") as pscpool,
            tc.tile_pool(name="ps2", bufs=1, space="PSUM") as ps2pool,
            tc.tile_pool(name="ps3", bufs=4, space="PSUM") as ps3pool,
        ):
            nc.gpsimd.load_library(library_config.mlp)

            # xcl chunk 0 first on SP (conv critical path); small consts on
            # the Activation HWDGE so they don't delay it
            s_xcl = vpool.tile([C, XCLM], F16)
            for ch in range(4):
                lo = (XCLM // 4) * ch
                hi = XCLM if ch == 3 else (XCLM // 4) * (ch + 1)
                nc.sync.dma_start(out=s_xcl[:, lo:hi], in_=xcl[:, lo:hi])
            s_offk = cpool.tile([C, K * 27], F16)
            nc.scalar.dma_start(out=s_offk[:], in_=offk[:])
            s_c16r = cpool.tile([1, 160], F16)
            nc.scalar.dma_start(out=s_c16r[:], in_=c16r[:])
            s_eye16 = cpool.tile([128, 128], F16)
            nc.scalar.dma_start(out=s_eye16[:], in_=eye16[:])
            s_const = cpool.tile([128, 51], F32)
            nc.scalar.dma_start(out=s_const[:], in_=consts[:])
            s_filt = cpool.tile([C, K * 2 * 128], F16)
            nc.sync.dma_start(out=s_filt[:], in_=filt[:])
            y_all = s_const[:, 0:32]          # [128, 32]
            dy_t = s_const[:, 32:41]          # [128, 9]
            dx_t = s_const[:, 41:50]
            x_all = s_const[:, 50:51]         # [128, 1]

            w16 = cpool.tile([128, 32, 9, 1, 4], F16)   # (k, y, x) weights
            idxw = cpool.tile([128, 2304], I16)         # wrapped gather indices
            idx_dram = dpool.tile([16, 2304], I16)

            wiT = bpool.tile([128, 32, 18], F32)

            # stage-B working tiles (written/consumed in halves)
            S = [128, 32, 9]
            sigm = bpool.tile(S, F32)
            py = bpool.tile(S, F32, tag="py")
            y0p = bpool.tile(S, F32, tag="y0p")
            fy = bpool.tile(S, F32, tag="fy")
            wy0 = bpool.tile(S, F32, tag="wy0")
            px = bpool.tile(S, F32, tag="px")
            x0p = bpool.tile(S, F32, tag="x0p")
            fx = bpool.tile(S, F32, tag="fx")
            wx0 = bpool.tile(S, F32, tag="wx0")
            qx = bpool.tile(S, F32, tag="qx")
            parx = bpool.tile(S, F32, tag="parx")
            qy = bpool.tile(S, F32, tag="qy")
            pary = bpool.tile(S, F32, tag="pary")
            base = bpool.tile(S, F32, tag="base")
            a0 = bpool.tile(S, F32, tag="a0")
            a1 = bpool.tile(S, F32, tag="a1")
            w_f32 = bpool.tile([128, 32, 9, 2, 2], F32)
            idx_i16 = bpool.tile([128, 32, 9], I16)

            def conv_group(bb):
                """Transposed offset conv for blocks 4bb..4bb+4: per block
                half-row r, out[pix, 27] = sum_t x_win_t^T @ offk_t + bias
                (stationary operand must have a single free dim on HW)."""
                pt3 = pscpool.tile([128, 4, 27], F32, tag="convps")
                for i in range(4):
                    b = 4 * bb + i
                    q0 = (2 * b + 1) * CONVW
                    for r in range(2):
                        for t in range(K):
                            d = int(DY[t]) * CONVW + int(DX[t])
                            a = 67 + q0 + d + 1 + r * CONVW
                            nc.tensor.matmul(
                                out=pt3[64 * r:64 * (r + 1), i, :],
                                lhsT=s_xcl[:, a:a + 64],
                                rhs=s_offk[:, t * 27:(t + 1) * 27],
                                start=(t == 0), stop=False,
                            )
                        nc.tensor.matmul(
                            out=pt3[64 * r:64 * (r + 1), i, :],
                            lhsT=s_c16r[0:1, 0:64],
                            rhs=s_c16r[0:1, 128:155], start=False, stop=True)
                # offsets out as-is; mask channels through sigmoid right here
                # so no Act-engine work downstream depends on the idx chain
                nc.scalar.copy(out=wiT[:, 4 * bb:4 * bb + 4, :],
                               in_=pt3[:, :, 0:18])
                nc.scalar.activation(sigm[:, 4 * bb:4 * bb + 4, :],
                                     pt3[:, :, 18:27],
                                     mybir.ActivationFunctionType.Sigmoid)

            def stage_b(lo, hi):
                """Coordinates, weights and gather indices for blocks
                [lo, hi)."""
                sl = (slice(None), slice(lo, hi))
                Sh = [128, hi - lo, 9]
                o1 = wiT[:, lo:hi, 0:9]
                o2 = wiT[:, lo:hi, 9:18]

                nc.vector.tensor_tensor(
                    out=py[sl], in0=o1, in1=bcast(y_all[:, lo:hi], Sh), op=AL.add)
                nc.vector.tensor_tensor(
                    out=py[sl], in0=py[sl],
                    in1=bcast(dy_t.rearrange("p (o k) -> p o k", o=1), Sh),
                    op=AL.add)
                nc.vector.tensor_scalar(out=py[sl], in0=py[sl], scalar1=8.0,
                                        scalar2=2.0, op0=AL.add, op1=AL.max)
                nc.vector.tensor_scalar(out=py[sl], in0=py[sl], scalar1=77.0,
                                        scalar2=None, op0=AL.min)
                nc.vector.tensor_scalar(out=y0p[sl], in0=py[sl], scalar1=-0.5,
                                        scalar2=8388608.0, op0=AL.add, op1=AL.add)
                nc.vector.tensor_scalar(out=y0p[sl], in0=y0p[sl],
                                        scalar1=-8388608.0, scalar2=None,
                                        op0=AL.add)
                nc.vector.tensor_tensor(out=fy[sl], in0=py[sl], in1=y0p[sl],
                                        op=AL.subtract)
                nc.vector.tensor_scalar(out=wy0[sl], in0=fy[sl], scalar1=-1.0,
                                        scalar2=1.0, op0=AL.mult, op1=AL.add)

                nc.vector.tensor_tensor(
                    out=px[sl], in0=o2, in1=bcast(x_all, Sh), op=AL.add)
                nc.vector.tensor_tensor(
                    out=px[sl], in0=px[sl],
                    in1=bcast(dx_t.rearrange("p (o k) -> p o k", o=1), Sh),
                    op=AL.add)
                nc.vector.tensor_scalar(out=px[sl], in0=px[sl], scalar1=8.0,
                                        scalar2=2.0, op0=AL.add, op1=AL.max)
                nc.vector.tensor_scalar(out=px[sl], in0=px[sl], scalar1=77.0,
                                        scalar2=None, op0=AL.min)
                nc.vector.tensor_scalar(out=x0p[sl], in0=px[sl], scalar1=-0.5,
                                        scalar2=8388608.0, op0=AL.add, op1=AL.add)
                nc.vector.tensor_scalar(out=x0p[sl], in0=x0p[sl],
                                        scalar1=-8388608.0, scalar2=None,
                                        op0=AL.add)
                nc.vector.tensor_tensor(out=fx[sl], in0=px[sl], in1=x0p[sl],
                                        op=AL.subtract)
                nc.vector.tensor_scalar(out=wx0[sl], in0=fx[sl], scalar1=-1.0,
                                        scalar2=1.0, op0=AL.mult, op1=AL.add)

                nc.vector.tensor_scalar(out=qx[sl], in0=x0p[sl], scalar1=0.5,
                                        scalar2=-0.25, op0=AL.mult, op1=AL.add)
                nc.vector.tensor_scalar(out=qx[sl], in0=qx[sl], scalar1=8388608.0,
                                        scalar2=-8388608.0, op0=AL.add,
                                        op1=AL.add)
                nc.vector.scalar_tensor_tensor(
                    out=parx[sl], in0=qx[sl], scalar=-2.0, in1=x0p[sl],
                    op0=AL.mult, op1=AL.add)
                nc.vector.tensor_scalar(out=qy[sl], in0=y0p[sl], scalar1=0.5,
                                        scalar2=-0.25, op0=AL.mult, op1=AL.add)
                nc.vector.tensor_scalar(out=qy[sl], in0=qy[sl], scalar1=8388608.0,
                                        scalar2=-8388608.0, op0=AL.add,
                                        op1=AL.add)
                nc.vector.scalar_tensor_tensor(
                    out=pary[sl], in0=qy[sl], scalar=-2.0, in1=y0p[sl],
                    op0=AL.mult, op1=AL.add)
                nc.vector.scalar_tensor_tensor(
                    out=base[sl], in0=qy[sl], scalar=40.0, in1=qx[sl],
                    op0=AL.mult, op1=AL.add)
                nc.vector.scalar_tensor_tensor(
                    out=base[sl], in0=parx[sl], scalar=1600.0, in1=base[sl],
                    op0=AL.mult, op1=AL.add)
                nc.vector.scalar_tensor_tensor(
                    out=base[sl], in0=pary[sl], scalar=3200.0, in1=base[sl],
                    op0=AL.mult, op1=AL.add)
                nc.vector.tensor_copy(out=idx_i16[sl], in_=base[sl])

                # scattered write into the wrapped-index DRAM staging:
                # wrapped[q, (g, pg)] = flatidx[(pg*16+q)*288 + g]
                idx_flat = idx_i16[:].rearrange("p b k -> p (b k)")
                c0, c1 = 9 * lo, 9 * hi
                for pg in range(8):
                    nc.sync.dma_start(
                        out=idx_dram[:, 8 * c0:8 * c1].rearrange(
                            "q (g pg) -> q g pg", pg=8)[:, :, pg],
                        in_=idx_flat[16 * pg:16 * (pg + 1), c0:c1])
                # one broadcast load replicates the quarter to all 8 groups
                nc.sync.dma_start(
                    out=idxw[:, 8 * c0:8 * c1].rearrange(
                        "(r q) g -> q r g", r=8),
                    in_=idx_dram[:, 8 * c0:8 * c1].rearrange(
                        "q (o g) -> q o g", o=1).to_broadcast(
                        [16, 8, 8 * (c1 - c0)]))

                # weights W [128, b, 9, 2, 2]  (k, y, x), mask folded in
                nc.vector.tensor_tensor(out=a0[sl], in0=wy0[sl], in1=sigm[sl],
                                        op=AL.mult)
                nc.vector.tensor_tensor(out=a1[sl], in0=fy[sl], in1=sigm[sl],
                                        op=AL.mult)
                nc.vector.tensor_tensor(out=w_f32[:, lo:hi, :, 0, 0],
                                        in0=a0[sl], in1=wx0[sl], op=AL.mult)
                nc.vector.tensor_tensor(out=w_f32[:, lo:hi, :, 0, 1],
                                        in0=a0[sl], in1=fx[sl], op=AL.mult)
                nc.vector.tensor_tensor(out=w_f32[:, lo:hi, :, 1, 0],
                                        in0=a1[sl], in1=wx0[sl], op=AL.mult)
                nc.vector.tensor_tensor(out=w_f32[:, lo:hi, :, 1, 1],
                                        in0=a1[sl], in1=fx[sl], op=AL.mult)
                nc.vector.tensor_copy(
                    out=w16[:, lo:hi].rearrange("p b k o u -> p b (k o u)"),
                    in_=w_f32[:, lo:hi].rearrange("p b k y u -> p b (k y u)"))

            def unit_compute(j):
                """Gather + weight + transpose + GEMM for a 2-block unit
                (18 slots, 256 output pixels)."""
                g0 = j * 18
                cols = colpool.tile([128, K, 256], F16, tag="cols")
                gws = []
                for bi in range(2):
                    b = 2 * j + bi
                    # one 9-slot gather per block: fine-grained dst recycling
                    dst = sgpool.tile([128, 9, 512], F16, tag="dst")
                    lo = g0 + 9 * bi
                    nc.gpsimd.dma_gather(
                        dst[:], pairs[:], idxw[:, lo * 8:(lo + 9) * 8],
                        9 * 128, 9 * 128, 512)
                    # weighted corners: dst row is [c, (y, x)] so the weight
                    # broadcast is mid-dim and the innermost stays packed
                    # fp16 (DVE 2x mode); split per tap-range so the PE can
                    # start on early taps while the rest lands
                    gw = blkpool.tile([128, K, 128, 4], F16, tag="gw")
                    dsrc = dst[:].rearrange("p s (c u) -> p s c u", u=4)
                    with tc.high_priority():
                        for t0, t1 in ((0, 5), (5, 9)):
                            nc.vector.tensor_tensor(
                                out=gw[:, t0:t1], in0=dsrc[:, t0:t1],
                                in1=bcast(w16[:, b, t0:t1],
                                          [128, t1 - t0, 128, 4]),
                                op=AL.mult)
                    gws.append(gw)
                # transpose-accumulate the 4 corners per (tap-pair, block);
                # copies stay off DVE/Pool (in-order queues pace the
                # multiplies and gather generation there)
                for k in range(0, K, 2):
                    kk = min(k + 2, K)
                    pc = ps3pool.tile([128, kk - k, 256], F32, tag="ctps")
                    for kt in range(k, kk):
                        for bi in range(2):
                            for u in range(4):
                                nc.tensor.matmul(
                                    out=pc[:, kt - k, bi * 128:(bi + 1) * 128],
                                    lhsT=gws[bi][:, kt, :, u],
                                    rhs=s_eye16[:],
                                    start=(u == 0), stop=(u == 3))
                    nc.scalar.copy(out=cols[:, k:kk, :], in_=pc[:])
                po0 = ps2pool.tile([128, 256], F32, tag="outps0")
                po1 = ps2pool.tile([128, 256], F32, tag="outps1")
                pos = [po0, po1]
                for k in range(K):
                    for fc in range(2):
                        nc.tensor.matmul(
                            out=pos[fc][:],
                            lhsT=s_filt[:, (k * 2 + fc) * 128:
                                        (k * 2 + fc + 1) * 128],
                            rhs=cols[:, k, :],
                            start=(k == 0), stop=(k == K - 1))
                for fc in range(2):
                    osb = blkpool.tile([128, 256], F16, tag="osb")
                    nc.scalar.copy(out=osb[:], in_=pos[fc][:])
                    nc.scalar.dma_start(
                        out=out_d[fc, :, j * 256:(j + 1) * 256], in_=osb[:])

            # PE warm-up: dummy matmuls ramp the tensor engine to full clock
            # before the offset conv lands
            warm = pscpool.tile([128, 27], F32, tag="convps")
            for i in range(24):
                nc.tensor.matmul(out=warm[:], lhsT=s_eye16[:], rhs=s_eye16[:, 0:27],
                                 start=(i == 0), stop=(i == 23))
            wsink = bpool.tile([128, 27], F32, tag="wsink")
            nc.vector.tensor_copy(out=wsink[:], in_=warm[:])

            # fast-start first unit; emit each next quarter's stage B midway
            # through the current quarter's units so the index tables are
            # ready without delaying the unit multiplies (DVE is in-order)
            conv_group(0)
            stage_b(0, 2)
            conv_group(1)
            stage_b(2, 8)
            unit_compute(0)
            unit_compute(1)
            conv_group(2)
            conv_group(3)
            stage_b(8, 16)
            unit_compute(2)
            unit_compute(3)
            unit_compute(4)
            unit_compute(5)
            conv_group(4)
            conv_group(5)
            stage_b(16, 24)
            for j in range(6, 10):
                unit_compute(j)
            conv_group(6)
            conv_group(7)
            stage_b(24, 32)
            for j in range(10, 16):
                unit_compute(j)
    nc.compile()
    return nc


def host_inputs(x, offset_kernel, offset_bias, filt_w):
    """Per-sample input maps. x [8,64,64,128] f32 etc (numpy)."""
    offk = np.ascontiguousarray(
        offset_kernel.reshape(K, C, 27).transpose(1, 0, 2).reshape(C, K * 27)
    ).astype(np.float16)
    c16r = np.zeros((1, 160), np.float16)
    c16r[0, 0:128] = 1.0
    c16r[0, 128:155] = offset_bias.astype(np.float16)
    filt_re = np.ascontiguousarray(
        filt_w.reshape(K, C, 2, 128).transpose(1, 0, 2, 3).reshape(C, K * 2 * 128)
    ).astype(np.float16)
    eye16 = np.eye(128).astype(np.float16)
    consts = np.zeros((128, 51), np.float32)
    p = np.arange(128)
    yoff = p // 64
    consts[:, 0:32] = 2 * np.arange(32)[None, :] + yoff[:, None]
    consts[:, 32:41] = DY[None, :]
    consts[:, 41:50] = DX[None, :]
    consts[:, 50] = p % 64

    maps = []
    for b in range(x.shape[0]):
        xp = np.zeros((HP + 2, WP + 2, C), np.float32)
        xp[PADR:PADR + H, PADR:PADR + W] = x[b]
        # rows channel-major, corner-minor: [pY, pX, cy, cx, C, uy, ux]
        quad = np.zeros((2, 2, 40, 40, C, 2, 2), np.float32)
        for pY in range(2):
            for pX in range(2):
                for uy in range(2):
                    for ux in range(2):
                        quad[pY, pX, :, :, :, uy, ux] = \
                            xp[pY + uy:pY + uy + 80:2, pX + ux:pX + ux + 80:2]
        prs = quad.reshape(NROWS, 4 * C).astype(np.float16)

        x1 = np.zeros((CONVW, CONVW, C), np.float32)
        x1[1:65, 1:65] = x[b]
        xcl = np.zeros((C, XCLM), np.float16)
        xcl[:, 67:67 + 4356] = x1.reshape(CONVW * CONVW, C).T.astype(np.float16)
        maps.append({
            "xcl": xcl, "pairs": prs, "offk": offk, "c16r": c16r,
            "filt": filt_re, "eye16": eye16, "consts": consts,
        })
    return maps


def host_output(res_list):
    outs = []
    for r in res_list:
        o = r["out"].reshape(256, NPIX).astype(np.float32)
        outs.append(np.ascontiguousarray(o.T).reshape(H, W, F))
    return np.stack(outs)


def _get_nc():
    global _NC
    if _NC is None:
        _NC = build_nc()
    return _NC


def kernel(inputs, offset_kernel, offset_bias, filt):
    from concourse.bass_utils import run_bass_kernel_spmd
    x = np.asarray(inputs, dtype=np.float32)
    maps = host_inputs(x, np.asarray(offset_kernel, np.float32),
                       np.asarray(offset_bias, np.float32),
                       np.asarray(filt, np.float32))
    nc = _get_nc()
    res = run_bass_kernel_spmd(nc, maps, core_ids=list(range(8)))
    return host_output(res.results).astype(np.float32)
